# revision 9
# baseline (speedup 1.0000x reference)
"""Causal multi-head self-attention (RoPE) Trainium2 Bass kernel.

Problem: x:(4,2048,1024), Wq/Wk/Wv:(1024,1024), Wo:(1024,1024), bo:(1024,)
  q,k,v = split_heads(x@W*), rope(q), rope(k), causal softmax(q k^T/8) v, @Wo+bo

Sharding: head-parallel across 8 cores. Core c owns heads {2c, 2c+1} for all
4 batches: it computes q/k/v projections against the 128-column weight slice,
attention for its heads, and a partial output projection against the matching
128-row slice of Wo. Host sums the 8 partial (8192,1024) outputs and adds bo.

On-core layout (all "T" tensors are feature-major: partitions=feature rows,
free=tokens):
  Q^T/K^T (128 x 2048/batch): rows = [h0 d-evens(32), h0 d-odds(32), h1 ...]
    (NeoX-style d-permutation, folded into the host-permuted weight columns;
     valid because q and k get the same permutation and qk^T is d-invariant)
  RoPE: Q <- Q*cos + (P2@Q)*sin2, where P2 swaps the even/odd halves per head
    (PE matmul) and sin2 carries the sign; 3 DVE passes per tensor-block.
  S^T tiles (tj x ti) = K^T.T @ Q^T per head (fp32r, K=64 contraction).
  A = exp(0.125*S^T) (ACT, straddle tiles band-masked with -1e30 triangle).
  O~^T (65 x ti) accumulated = [V|1].T @ A over tj chunks; row 64 = softmax
    denominators (ones column trick). Normalize via ACT reciprocal +
    DRAM-staged partition broadcast + DVE multiply -> O^T (128 x 2048).
  y partial (128t x 1024) = O^T-chunk.T @ Wo-slice, DMA'd psum->DRAM.
"""

import numpy as np

B, T, C = 4, 2048, 1024
H, D = 16, 64
N_CORES = 8
BT = B * T
SCALE = 0.125  # D**-0.5
NEG = -1.0e30

TRACE = False            # set True (e.g. from test.py) to capture an NTFF trace
LAST_RESULT = None       # BassKernelResults of the most recent run

_BUILT = None            # cached (nc, input-name list)


# --------------------------------------------------------------------------
# workaround: this walrus build rejects >1 semaphore wait per instruction
def _split_sem_waits(nc, max_waits=1):
    import concourse.mybir as mybir

    n = 0
    for f in nc.m.functions:
        for bb in f.blocks:
            insts = bb.instructions
            idx = 0
            while idx < len(insts):
                i = insts[idx]
                si = getattr(i, "sync_info", None)
                if si is not None and si.on_wait and len(si.on_wait) > max_waits:
                    waits = list(si.on_wait)
                    extra, keep = waits[:-max_waits], waits[-max_waits:]
                    si.on_wait = keep
                    pos = idx
                    for j in range(0, len(extra), max_waits):
                        n += 1
                        nd = mybir.InstNoOp(name=f"I-waitsplit-{n}", ins=[], outs=[])
                        nd.engine = i.engine
                        nd.sync_info = mybir.SyncInfo(
                            on_wait=extra[j : j + max_waits], on_update=[]
                        )
                        insts.insert(pos, nd)
                        pos += 1
                    idx = pos
                idx += 1


def _install_ntff_hook():
    """The image's antenv lacks axon_hooks; synthesize it so trace=True works."""
    import sys
    import types

    if "antenv.axon_hooks" in sys.modules:
        return
    import antenv

    state = {"hook": None}
    mod = types.ModuleType("antenv.axon_hooks")
    mod.get_axon_ntff_profile_hook = lambda: state["hook"]
    mod.set_axon_ntff_profile_hook = lambda h: state.__setitem__("hook", h)
    sys.modules["antenv.axon_hooks"] = mod
    antenv.axon_hooks = mod
    try:
        from trn_agent_boot.trn_boot import _ntff_profile_via_ctypes

        state["hook"] = _ntff_profile_via_ctypes("/opt/axon/libaxon_pjrt.so")
    except Exception:
        state["hook"] = None


# --------------------------------------------------------------------------
def _build():
    import concourse.bass as bass
    import concourse.mybir as mybir
    from concourse.tile import TileContext

    F = mybir.dt.float32
    FR = mybir.dt.float32r
    MULT = mybir.AluOpType.mult
    ADD = mybir.AluOpType.add
    EXP = mybir.ActivationFunctionType.Exp
    RCP = mybir.ActivationFunctionType.Reciprocal

    nc = bass.Bass()

    xT = nc.dram_tensor("xT", (C, BT), FR, kind="ExternalInput")
    wq = nc.dram_tensor("wq", (C, 128), FR, kind="ExternalInput")
    wk = nc.dram_tensor("wk", (C, 128), FR, kind="ExternalInput")
    wv = nc.dram_tensor("wv", (C, 128), FR, kind="ExternalInput")
    wo = nc.dram_tensor("wo", (128, C), FR, kind="ExternalInput")
    cosd = nc.dram_tensor("cos", (128, T), F, kind="ExternalInput")
    sind = nc.dram_tensor("sin2", (128, T), F, kind="ExternalInput")
    p2d = nc.dram_tensor("p2", (128, 128), FR, kind="ExternalInput")
    bandd = nc.dram_tensor("band", (128, 128), F, kind="ExternalInput")
    id2d = nc.dram_tensor("id2", (128, 64), F, kind="ExternalInput")
    vonesd = nc.dram_tensor("vones", (128, 32), FR, kind="ExternalInput")
    y = nc.dram_tensor("y", (BT, C), F, kind="ExternalOutput")
    scr = nc.dram_tensor("scr", (B * 8, 512), F, kind="Internal")

    with TileContext(nc) as tc:
        with (
            tc.tile_pool(name="const", bufs=1) as cst,
            tc.tile_pool(name="xt", bufs=2) as xtp,
            tc.tile_pool(name="qt", bufs=2) as qp,
            tc.tile_pool(name="kt", bufs=2) as kp,
            tc.tile_pool(name="vt", bufs=2) as vp,
            tc.tile_pool(name="ot", bufs=2) as op_,
            tc.tile_pool(name="vst", bufs=2) as vstp,
            tc.tile_pool(name="tmp", bufs=3) as tmp,
            tc.tile_pool(name="at", bufs=4) as ap_,
            tc.tile_pool(name="bc", bufs=2) as bcp,
            tc.tile_pool(name="rr", bufs=2) as rp,
            tc.tile_pool(name="ys", bufs=3) as ysp,
            tc.tile_pool(name="sps", bufs=4, space="PSUM") as sps,
            tc.tile_pool(name="avp", bufs=2, space="PSUM") as avp,
            tc.tile_pool(name="yp", bufs=1, space="PSUM") as yp,
        ):
            # ---- constants -------------------------------------------------
            wq_t = cst.tile([128, 8, 128], FR)
            wk_t = cst.tile([128, 8, 128], FR)
            wv_t = cst.tile([128, 8, 128], FR)
            for k in range(8):
                nc.sync.dma_start(out=wq_t[:, k, :], in_=wq[k * 128 : (k + 1) * 128, :])
                nc.sync.dma_start(out=wk_t[:, k, :], in_=wk[k * 128 : (k + 1) * 128, :])
                nc.sync.dma_start(out=wv_t[:, k, :], in_=wv[k * 128 : (k + 1) * 128, :])
            wo_t = cst.tile([128, C], FR)
            nc.sync.dma_start(out=wo_t, in_=wo[:, :])
            cos_t = cst.tile([128, T], F)
            nc.sync.dma_start(out=cos_t, in_=cosd[:, :])
            sin_t = cst.tile([128, T], F)
            nc.sync.dma_start(out=sin_t, in_=sind[:, :])
            p2_t = cst.tile([128, 128], FR)
            nc.sync.dma_start(out=p2_t, in_=p2d[:, :])
            band_t = cst.tile([128, 128], F)
            nc.sync.dma_start(out=band_t, in_=bandd[:, :])
            id_t = cst.tile([128, 64], F)
            nc.sync.dma_start(out=id_t, in_=id2d[:, :])

            for b in range(B):
                # ---- phase A/B/C: projections + rope + V transpose --------
                Qb = qp.tile([128, T], FR)
                Kb = kp.tile([128, T], FR)
                Vb = vp.tile([128, 16, 130], FR)  # [tj-tile][h*65+d], col 64=ones
                nc.sync.dma_start(
                    out=Vb[:, :, 64:130:65],
                    in_=vonesd[:, :].rearrange("p (a b) -> p a b", b=2),
                )
                for nb in range(4):
                    g0 = b * T + nb * 512
                    cols = slice(nb * 512, (nb + 1) * 512)
                    xt = xtp.tile([128, 8, 512], FR)
                    for k in range(8):
                        nc.sync.dma_start(
                            out=xt[:, k, :],
                            in_=xT[k * 128 : (k + 1) * 128, g0 : g0 + 512],
                        )
                    for W, dst, rope in ((wq_t, Qb, True), (wk_t, Kb, True),
                                         (wv_t, None, False)):
                        ps = sps.tile([128, 512], F, tag="s")
                        for k in range(8):
                            nc.tensor.matmul(
                                ps[:, :], lhsT=W[:, k, :], rhs=xt[:, k, :],
                                start=(k == 0), stop=(k == 7),
                            )
                        if rope:
                            nc.scalar.copy(dst[:, cols], ps[:, :])
                            rot = sps.tile([128, 512], F, tag="s")
                            nc.tensor.matmul(
                                rot[:, :], lhsT=p2_t[:, :],
                                rhs=dst[:, cols], start=True, stop=True,
                            )
                            t1 = tmp.tile([128, 512], F)
                            nc.vector.tensor_tensor(
                                t1[:, :], rot[:, :], sin_t[:, cols], MULT)
                            nc.vector.tensor_tensor(
                                dst[:, cols], dst[:, cols], cos_t[:, cols], MULT)
                            nc.vector.tensor_tensor(
                                dst[:, cols], dst[:, cols], t1[:, :], ADD)
                        else:
                            vst = vstp.tile([128, 512], F)
                            nc.scalar.copy(vst[:, :], ps[:, :])
                            for tl in range(4):
                                tt = nb * 4 + tl
                                tcs = slice(tl * 128, (tl + 1) * 128)
                                for h in (0, 1):
                                    tp = sps.tile([128, 64], F, tag="s")
                                    nc.tensor.transpose(
                                        tp[:, :], vst[64 * h : 64 * h + 64, tcs],
                                        id_t[64 * h : 64 * h + 64, :],
                                    )
                                    nc.vector.tensor_copy(
                                        Vb[:, tt, 65 * h : 65 * h + 64], tp[:, :])

                # ---- phase D: attention -----------------------------------
                Ob = op_.tile([128, T], FR)
                for i in range(4):
                    av = [avp.tile([128, 512], F, tag="av", name="av")
                          for _ in (0, 1)]
                    nch = 4 * i + 4
                    for j in range(nch):
                        off = max(0, j * 128 - i * 512)
                        nl = 512 - off
                        for h in (0, 1):
                            hs = slice(64 * h, 64 * h + 64)
                            st = sps.tile([128, 512], F, tag="s")
                            nc.tensor.matmul(
                                st[:, 0:nl],
                                lhsT=Kb[hs, j * 128 : (j + 1) * 128],
                                rhs=Qb[hs, i * 512 + off : (i + 1) * 512],
                                start=True, stop=True,
                            )
                            if j >= 4 * i:  # straddles the diagonal
                                nc.vector.tensor_tensor(
                                    st[:, 0:128], st[:, 0:128], band_t[:, :], ADD)
                            A = ap_.tile([128, 512], FR)
                            nc.scalar.activation(
                                A[:, 0:nl], st[:, 0:nl], EXP, scale=SCALE)
                            nc.tensor.matmul(
                                av[h][0:65, off:512],
                                lhsT=Vb[:, j, 65 * h : 65 * h + 65],
                                rhs=A[:, 0:nl],
                                start=(j == 0), stop=(j == nch - 1),
                                skip_group_check=True,
                            )
                    for h in (0, 1):
                        rt = rp.tile([1, 512], F)
                        nc.vector.reciprocal(rt[0:1, :], av[h][64:65, :])
                        row = b * 8 + i * 2 + h
                        nc.sync.dma_start(out=scr[row : row + 1, :], in_=rt[0:1, :])
                        bct = bcp.tile([64, 512], F)
                        src = scr[row : row + 1, :]
                        bap = bass.AP(
                            tensor=src.tensor, offset=src.offset,
                            ap=[[0, 64]] + [list(p) for p in src.ap[1:]],
                        )
                        nc.sync.dma_start(out=bct[:, :], in_=bap)
                        nc.vector.tensor_tensor(
                            Ob[64 * h : 64 * h + 64, i * 512 : (i + 1) * 512],
                            av[h][0:64, :], bct[:, :], MULT,
                        )

                # ---- phase E: partial output projection -------------------
                for tt in range(16):
                    yps = yp.tile([128, 1024], F)
                    lhs = Ob[:, tt * 128 : (tt + 1) * 128]
                    for nh in (0, 1):
                        nc.tensor.matmul(
                            yps[:, nh * 512 : (nh + 1) * 512],
                            lhsT=lhs, rhs=wo_t[:, nh * 512 : (nh + 1) * 512],
                            start=True, stop=True,
                        )
                    ysb = ysp.tile([128, 1024], F)
                    nc.vector.tensor_copy(ysb[:, 0:512], yps[:, 0:512])
                    nc.scalar.copy(ysb[:, 512:1024], yps[:, 512:1024])
                    r0 = b * T + tt * 128
                    nc.sync.dma_start(out=y[r0 : r0 + 128, :], in_=ysb[:, :])

    _split_sem_waits(nc)
    return nc


# --------------------------------------------------------------------------
def _host_inputs(x, Wq, Wk, Wv):
    """Per-core input dicts (all shared arrays built once)."""
    xT = np.ascontiguousarray(np.asarray(x, dtype=np.float32).reshape(BT, C).T)

    # NeoX d-permutation within each head: evens then odds
    dperm = np.concatenate([np.arange(0, D, 2), np.arange(1, D, 2)])

    inv_freq = (1.0 / (10000.0 ** (np.arange(0, D, 2) / D))).astype(np.float64)
    pos = np.arange(T, dtype=np.float64)
    ang = pos[None, :] * inv_freq[:, None]  # (32, T)
    cos32 = np.cos(ang).astype(np.float32)
    sin32 = np.sin(ang).astype(np.float32)
    cos_t = np.tile(np.vstack([cos32, cos32]), (2, 1))  # (128, T)
    sin_t = np.tile(np.vstack([-sin32, sin32]), (2, 1))  # (128, T), sign folded

    p2 = np.zeros((128, 128), dtype=np.float32)
    for hb in (0, 64):
        for i2 in range(32):
            p2[hb + i2, hb + 32 + i2] = 1.0
            p2[hb + 32 + i2, hb + i2] = 1.0

    band = np.where(
        np.arange(128)[None, :] < np.arange(128)[:, None], np.float32(NEG), 0.0
    ).astype(np.float32)
    id2 = np.tile(np.eye(D, dtype=np.float32), (2, 1))  # (128, 64)

    Wq = np.asarray(Wq, dtype=np.float32)
    Wk = np.asarray(Wk, dtype=np.float32)
    Wv = np.asarray(Wv, dtype=np.float32)

    in_maps = []
    for c in range(N_CORES):
        sl = slice(128 * c, 128 * (c + 1))
        wq_c = Wq[:, sl].reshape(C, 2, D)[:, :, dperm].reshape(C, 128)
        wk_c = Wk[:, sl].reshape(C, 2, D)[:, :, dperm].reshape(C, 128)
        in_maps.append({
            "xT": xT,
            "wq": np.ascontiguousarray(wq_c),
            "wk": np.ascontiguousarray(wk_c),
            "wv": np.ascontiguousarray(Wv[:, sl]),
            "wo": None,  # set below
            "cos": cos_t,
            "sin2": sin_t,
            "p2": p2,
            "band": band,
            "id2": id2,
            "vones": np.ones((128, 32), dtype=np.float32),
        })
    return in_maps


def kernel(x, Wq, Wk, Wv, Wo, bo):
    global _BUILT, LAST_RESULT
    from concourse.bass_utils import run_bass_kernel_spmd

    if TRACE:
        _install_ntff_hook()

    if _BUILT is None:
        _BUILT = _build()
    nc = _BUILT

    in_maps = _host_inputs(x, Wq, Wk, Wv)
    Wo = np.asarray(Wo, dtype=np.float32)
    for c in range(N_CORES):
        in_maps[c]["wo"] = np.ascontiguousarray(Wo[128 * c : 128 * (c + 1), :])

    res = run_bass_kernel_spmd(
        nc, in_maps, core_ids=list(range(N_CORES)), trace=TRACE
    )
    LAST_RESULT = res

    acc = res.results[0]["y"].astype(np.float64)
    for c in range(1, N_CORES):
        acc = acc + res.results[c]["y"]
    out = acc.astype(np.float32) + np.asarray(bo, dtype=np.float32)[None, :]
    return out.reshape(B, T, C)


# revision 17
# speedup vs baseline: 1.3050x; 1.3050x over previous
"""Causal multi-head self-attention (RoPE) Trainium2 Bass kernel.

Problem: x:(4,2048,1024), Wq/Wk/Wv:(1024,1024), Wo:(1024,1024), bo:(1024,)
  q,k,v = split_heads(x@W*), rope(q), rope(k), causal softmax(q k^T/8) v, @Wo+bo

Sharding: head-parallel across 8 cores. Core c owns heads {2c, 2c+1} for all
4 batches: it computes q/k/v projections against the 128-column weight slice,
attention for its heads, and a partial output projection against the matching
128-row slice of Wo. Host sums the 8 partial (8192,1024) outputs and adds bo.

On-core layout (all "T" tensors are feature-major: partitions=feature rows,
free=tokens):
  Q^T/K^T (128 x 2048/batch): rows = [h0 d-evens(32), h0 d-odds(32), h1 ...]
    (NeoX-style d-permutation, folded into the host-permuted weight columns;
     valid because q and k get the same permutation and qk^T is d-invariant)
  RoPE: Q <- Q*cos + (P2@Q)*sin2, where P2 swaps the even/odd halves per head
    (PE matmul) and sin2 carries the sign; 3 DVE passes per tensor-block.
  S^T tiles (tj x ti) = K^T.T @ Q^T per head (fp32r, K=64 contraction).
  A = exp(0.125*S^T) (ACT, straddle tiles band-masked with -1e30 triangle).
  O~^T (65 x ti) accumulated = [V|1].T @ A over tj chunks; row 64 = softmax
    denominators (ones column trick). Normalize via ACT reciprocal +
    DRAM-staged partition broadcast + DVE multiply -> O^T (128 x 2048).
  y partial (128t x 1024) = O^T-chunk.T @ Wo-slice, DMA'd psum->DRAM.
"""

import numpy as np

B, T, C = 4, 2048, 1024
H, D = 16, 64
N_CORES = 8
BT = B * T
SCALE = 0.125  # D**-0.5
NEG = -1.0e30

TRACE = False            # set True (e.g. from test.py) to capture an NTFF trace
LAST_RESULT = None       # BassKernelResults of the most recent run

_BUILT = None            # cached (nc, input-name list)


# --------------------------------------------------------------------------
# workaround: this walrus build rejects >1 semaphore wait per instruction
def _split_sem_waits(nc, max_waits=1):
    import concourse.mybir as mybir

    n = 0
    for f in nc.m.functions:
        for bb in f.blocks:
            insts = bb.instructions
            idx = 0
            while idx < len(insts):
                i = insts[idx]
                si = getattr(i, "sync_info", None)
                if si is not None and si.on_wait and len(si.on_wait) > max_waits:
                    waits = list(si.on_wait)
                    extra, keep = waits[:-max_waits], waits[-max_waits:]
                    si.on_wait = keep
                    pos = idx
                    for j in range(0, len(extra), max_waits):
                        n += 1
                        nd = mybir.InstNoOp(name=f"I-waitsplit-{n}", ins=[], outs=[])
                        nd.engine = i.engine
                        nd.sync_info = mybir.SyncInfo(
                            on_wait=extra[j : j + max_waits], on_update=[]
                        )
                        insts.insert(pos, nd)
                        pos += 1
                    idx = pos
                idx += 1


def _install_ntff_hook():
    """The image's antenv lacks axon_hooks; synthesize it so trace=True works."""
    import sys
    import types

    if "antenv.axon_hooks" in sys.modules:
        return
    import antenv

    state = {"hook": None}
    mod = types.ModuleType("antenv.axon_hooks")
    mod.get_axon_ntff_profile_hook = lambda: state["hook"]
    mod.set_axon_ntff_profile_hook = lambda h: state.__setitem__("hook", h)
    sys.modules["antenv.axon_hooks"] = mod
    antenv.axon_hooks = mod
    try:
        from trn_agent_boot.trn_boot import _ntff_profile_via_ctypes

        state["hook"] = _ntff_profile_via_ctypes("/opt/axon/libaxon_pjrt.so")
    except Exception:
        state["hook"] = None


# --------------------------------------------------------------------------
def _build():
    import concourse.bass as bass
    import concourse.mybir as mybir
    from concourse.tile import TileContext

    F = mybir.dt.float32
    FR = mybir.dt.float32r
    MULT = mybir.AluOpType.mult
    ADD = mybir.AluOpType.add
    SUB = mybir.AluOpType.subtract
    EXP = mybir.ActivationFunctionType.Exp
    RCP = mybir.ActivationFunctionType.Reciprocal

    nc = bass.Bass()

    xT = nc.dram_tensor("xT", (C, BT), FR, kind="ExternalInput")
    wq = nc.dram_tensor("wq", (C, 128), FR, kind="ExternalInput")
    wk = nc.dram_tensor("wk", (C, 128), FR, kind="ExternalInput")
    wv = nc.dram_tensor("wv", (C, 128), FR, kind="ExternalInput")
    wo = nc.dram_tensor("wo", (128, C), FR, kind="ExternalInput")
    cosd = nc.dram_tensor("cos", (128, T), F, kind="ExternalInput")
    sind = nc.dram_tensor("sin2", (128, T), F, kind="ExternalInput")
    p2d = nc.dram_tensor("p2", (128, 128), FR, kind="ExternalInput")
    bandd = nc.dram_tensor("band2x", (128, 256), F, kind="ExternalInput")
    mf4d = nc.dram_tensor("mf4", (128, 512), F, kind="ExternalInput")
    id2d = nc.dram_tensor("id2", (128, 64), F, kind="ExternalInput")
    vonesd = nc.dram_tensor("vones", (128, 32), FR, kind="ExternalInput")
    y = nc.dram_tensor("y", (BT, C), F, kind="ExternalOutput")
    scr_s = nc.dram_tensor("scr_s", (B * 8, 512), F, kind="Internal")
    scr = nc.dram_tensor("scr", (B * 8, 512), F, kind="Internal")

    with TileContext(nc) as tc:
        with (
            tc.tile_pool(name="const", bufs=1) as cst,
            tc.tile_pool(name="xt", bufs=2) as xtp,
            tc.tile_pool(name="qt", bufs=2) as qp,
            tc.tile_pool(name="kt", bufs=2) as kp,
            tc.tile_pool(name="vt", bufs=2) as vp,
            tc.tile_pool(name="ot", bufs=2) as op_,
            tc.tile_pool(name="vst", bufs=2) as vstp,
            tc.tile_pool(name="tmp", bufs=3) as tmp,
            tc.tile_pool(name="at", bufs=4) as ap_,
            tc.tile_pool(name="bc", bufs=2) as bcp,
            tc.tile_pool(name="rr", bufs=2) as rp,
            tc.tile_pool(name="ys", bufs=3) as ysp,
            tc.tile_pool(name="sps", bufs=2, space="PSUM") as sps,
            tc.tile_pool(name="stp", bufs=2, space="PSUM") as stp,
            tc.tile_pool(name="avp", bufs=2, space="PSUM") as avp,
        ):
            # ---- constants -------------------------------------------------
            wq_t = cst.tile([128, 8, 128], FR)
            wk_t = cst.tile([128, 8, 128], FR)
            wv_t = cst.tile([128, 8, 128], FR)
            for k in range(8):
                nc.sync.dma_start(out=wq_t[:, k, :], in_=wq[k * 128 : (k + 1) * 128, :])
                nc.sync.dma_start(out=wk_t[:, k, :], in_=wk[k * 128 : (k + 1) * 128, :])
                nc.sync.dma_start(out=wv_t[:, k, :], in_=wv[k * 128 : (k + 1) * 128, :])
            wo_t = cst.tile([128, C], FR)
            nc.sync.dma_start(out=wo_t, in_=wo[:, :])
            cos_t = cst.tile([128, T], F)
            nc.sync.dma_start(out=cos_t, in_=cosd[:, :])
            sin_t = cst.tile([128, T], F)
            nc.sync.dma_start(out=sin_t, in_=sind[:, :])
            p2_t = cst.tile([128, 128], FR)
            nc.sync.dma_start(out=p2_t, in_=p2d[:, :])
            band_t = cst.tile([128, 256], F)  # [band | band] for head pairs
            nc.sync.dma_start(out=band_t, in_=bandd[:, :])
            mf4_t = cst.tile([128, 512], F)   # [full|band|full|band] (delta=384)
            nc.sync.dma_start(out=mf4_t, in_=mf4d[:, :])
            id_t = cst.tile([128, 64], F)
            nc.sync.dma_start(out=id_t, in_=id2d[:, :])

            QKV = {}  # b -> (Qb, Kb, Vb);  O = {} b -> Ob

            def phase_a(b):
                Qb = qp.tile([128, T], FR, name="Qb")
                Kb = kp.tile([128, T], FR, name="Kb")
                Vb = vp.tile([128, 16, 130], FR, name="Vb")  # col 64/129 = ones
                QKV[b] = (Qb, Kb, Vb)
                nc.sync.dma_start(
                    out=Vb[:, :, 64:130:65],
                    in_=vonesd[:, :].rearrange("p (a b) -> p a b", b=2),
                )
                for nb in range(4):
                    g0 = b * T + nb * 512
                    cols = slice(nb * 512, (nb + 1) * 512)
                    xt = xtp.tile([128, 8, 512], FR, name="xt")
                    for k in range(8):
                        nc.sync.dma_start(
                            out=xt[:, k, :],
                            in_=xT[k * 128 : (k + 1) * 128, g0 : g0 + 512],
                        )
                    for W, dst in ((wq_t, Qb), (wk_t, Kb)):
                        ps = sps.tile([128, 512], F, tag="s", name="ps")
                        for k in range(8):
                            nc.tensor.matmul(
                                ps[:, :], lhsT=W[:, k, :], rhs=xt[:, k, :],
                                start=(k == 0), stop=(k == 7),
                            )
                        # rope: dst = ps*cos - P2@(ps*sin2)
                        #   (P2@ (q.sin2))[p] = -q~[p]*sin2[p], since sin2 is
                        #    antisymmetric and cos symmetric under the pair swap
                        qs = tmp.tile([128, 512], FR, name="qs")
                        nc.vector.tensor_tensor(qs[:, :], ps[:, :],
                                                sin_t[:, cols], MULT)
                        nc.vector.tensor_tensor(dst[:, cols], ps[:, :],
                                                cos_t[:, cols], MULT)
                        rot = sps.tile([128, 512], F, tag="s", name="rot")
                        nc.tensor.matmul(rot[:, :], lhsT=p2_t[:, :], rhs=qs[:, :],
                                         start=True, stop=True)
                        nc.vector.tensor_tensor(dst[:, cols], dst[:, cols],
                                                rot[:, :], SUB)
                    ps = sps.tile([128, 512], F, tag="s", name="ps")
                    for k in range(8):
                        nc.tensor.matmul(
                            ps[:, :], lhsT=wv_t[:, k, :], rhs=xt[:, k, :],
                            start=(k == 0), stop=(k == 7),
                        )
                    vst = vstp.tile([128, 512], F, name="vst")
                    nc.scalar.copy(vst[:, :], ps[:, :])
                    for tl in range(4):
                        tt = nb * 4 + tl
                        tcs = slice(tl * 128, (tl + 1) * 128)
                        for h in (0, 1):
                            tp = sps.tile([128, 64], F, tag="s", name="tp")
                            nc.tensor.transpose(
                                tp[:, :], vst[64 * h : 64 * h + 64, tcs],
                                id_t[64 * h : 64 * h + 64, :],
                            )
                            nc.vector.tensor_copy(
                                Vb[:, tt, 65 * h : 65 * h + 64], tp[:, :])

            O = {}

            def phase_d(b):
                Qb, Kb, Vb = QKV[b]
                Ob = op_.tile([128, T], FR, name="Ob")
                O[b] = Ob
                for i in range(4):
                    av = [avp.tile([128, 512], F, tag="av", name="av")
                          for _ in (0, 1)]
                    nch = 4 * i + 4
                    for j in range(nch):
                        delta = j * 128 - i * 512
                        nl = 512 if delta < 0 else max(512 - delta, 256)
                        off = 512 - nl
                        st = stp.tile([128, 2, 512], F, name="st")
                        for h in (0, 1):
                            hs = slice(64 * h, 64 * h + 64)
                            nc.tensor.matmul(
                                st[:, h, 0:nl],
                                lhsT=Kb[hs, j * 128 : (j + 1) * 128],
                                rhs=Qb[hs, i * 512 + off : (i + 1) * 512],
                                start=True, stop=True,
                            )
                        if delta >= 0:
                            if delta == 384:  # [128 fully-masked | 128 band]
                                nc.vector.tensor_tensor(
                                    st[:, :, 0:256], st[:, :, 0:256],
                                    mf4_t[:, :].rearrange("p (a c) -> p a c", a=2),
                                    ADD)
                            else:
                                c0 = delta - off
                                nc.vector.tensor_tensor(
                                    st[:, :, c0 : c0 + 128], st[:, :, c0 : c0 + 128],
                                    band_t[:, :].rearrange("p (a c) -> p a c", a=2),
                                    ADD)
                        A = ap_.tile([128, 2, 512], FR, name="A")
                        nc.scalar.activation(
                            A[:, :, 0:nl], st[:, :, 0:nl], EXP, scale=SCALE)
                        for h in (0, 1):
                            nc.tensor.matmul(
                                av[h][0:65, off:512],
                                lhsT=Vb[:, j, 65 * h : 65 * h + 65],
                                rhs=A[:, h, 0:nl],
                                start=(j == 0), stop=(j == nch - 1),
                                skip_group_check=True,
                            )
                    for h in (0, 1):
                        row = b * 8 + i * 2 + h
                        # sums row -> DRAM -> (128x4) repartition -> lane-
                        # parallel reciprocal -> DRAM -> 64-row broadcast
                        sr = rp.tile([1, 512], F, name="sr")
                        nc.vector.tensor_copy(sr[0:1, :], av[h][64:65, :])
                        nc.sync.dma_start(out=scr_s[row : row + 1, :], in_=sr[0:1, :])
                        srt = rp.tile([128, 4], F, name="srt")
                        nc.sync.dma_start(
                            out=srt[:, :],
                            in_=scr_s[row : row + 1, :].rearrange(
                                "r (p c) -> (r p) c", c=4),
                        )
                        rt = rp.tile([128, 4], F, name="rt")
                        nc.vector.reciprocal(rt[:, :], srt[:, :])
                        nc.sync.dma_start(
                            out=scr[row : row + 1, :].rearrange(
                                "r (p c) -> (r p) c", c=4),
                            in_=rt[:, :],
                        )
                        bct = bcp.tile([64, 512], F, name="bct")
                        src = scr[row : row + 1, :]
                        bap = bass.AP(
                            tensor=src.tensor, offset=src.offset,
                            ap=[[0, 64]] + [list(p) for p in src.ap[1:]],
                        )
                        nc.sync.dma_start(out=bct[:, :], in_=bap)
                        nc.vector.tensor_tensor(
                            Ob[64 * h : 64 * h + 64, i * 512 : (i + 1) * 512],
                            av[h][0:64, :], bct[:, :], MULT,
                        )

            def phase_e(b):
                Ob = O.pop(b)
                for tt in range(16):
                    lhs = Ob[:, tt * 128 : (tt + 1) * 128]
                    ysb = ysp.tile([128, 1024], F, name="ysb")
                    for nh in (0, 1):
                        yps = avp.tile([128, 512], F, tag="av", name="yps")
                        nc.tensor.matmul(
                            yps[:, :], lhsT=lhs,
                            rhs=wo_t[:, nh * 512 : (nh + 1) * 512],
                            start=True, stop=True,
                        )
                        if nh == 0:
                            nc.vector.tensor_copy(ysb[:, 0:512], yps[:, :])
                        else:
                            nc.scalar.copy(ysb[:, 512:1024], yps[:, :])
                    r0 = b * T + tt * 128
                    nc.sync.dma_start(out=y[r0 : r0 + 128, :], in_=ysb[:, :])

            phase_a(0)
            for b in range(B):
                phase_d(b)
                if b + 1 < B:
                    phase_a(b + 1)
                phase_e(b)

    _split_sem_waits(nc)
    return nc


# --------------------------------------------------------------------------
def _host_inputs(x, Wq, Wk, Wv):
    """Per-core input dicts (all shared arrays built once)."""
    xT = np.ascontiguousarray(np.asarray(x, dtype=np.float32).reshape(BT, C).T)

    # NeoX d-permutation within each head: evens then odds
    dperm = np.concatenate([np.arange(0, D, 2), np.arange(1, D, 2)])

    inv_freq = (1.0 / (10000.0 ** (np.arange(0, D, 2) / D))).astype(np.float64)
    pos = np.arange(T, dtype=np.float64)
    ang = pos[None, :] * inv_freq[:, None]  # (32, T)
    cos32 = np.cos(ang).astype(np.float32)
    sin32 = np.sin(ang).astype(np.float32)
    cos_t = np.tile(np.vstack([cos32, cos32]), (2, 1))  # (128, T)
    sin_t = np.tile(np.vstack([-sin32, sin32]), (2, 1))  # (128, T), sign folded

    p2 = np.zeros((128, 128), dtype=np.float32)
    for hb in (0, 64):
        for i2 in range(32):
            p2[hb + i2, hb + 32 + i2] = 1.0
            p2[hb + 32 + i2, hb + i2] = 1.0

    band = np.where(
        np.arange(128)[None, :] < np.arange(128)[:, None], np.float32(NEG), 0.0
    ).astype(np.float32)
    band2x = np.concatenate([band, band], axis=1)  # (128, 256)
    full = np.full((128, 128), np.float32(NEG), dtype=np.float32)
    mf4 = np.concatenate([full, band, full, band], axis=1)  # (128, 512)
    id2 = np.tile(np.eye(D, dtype=np.float32), (2, 1))  # (128, 64)

    Wq = np.asarray(Wq, dtype=np.float32)
    Wk = np.asarray(Wk, dtype=np.float32)
    Wv = np.asarray(Wv, dtype=np.float32)

    in_maps = []
    for c in range(N_CORES):
        sl = slice(128 * c, 128 * (c + 1))
        wq_c = Wq[:, sl].reshape(C, 2, D)[:, :, dperm].reshape(C, 128)
        wk_c = Wk[:, sl].reshape(C, 2, D)[:, :, dperm].reshape(C, 128)
        in_maps.append({
            "xT": xT,
            "wq": np.ascontiguousarray(wq_c),
            "wk": np.ascontiguousarray(wk_c),
            "wv": np.ascontiguousarray(Wv[:, sl]),
            "wo": None,  # set below
            "cos": cos_t,
            "sin2": sin_t,
            "p2": p2,
            "band2x": band2x,
            "mf4": mf4,
            "id2": id2,
            "vones": np.ones((128, 32), dtype=np.float32),
        })
    return in_maps


def kernel(x, Wq, Wk, Wv, Wo, bo):
    global _BUILT, LAST_RESULT
    from concourse.bass_utils import run_bass_kernel_spmd

    if TRACE:
        _install_ntff_hook()

    if _BUILT is None:
        _BUILT = _build()
    nc = _BUILT

    in_maps = _host_inputs(x, Wq, Wk, Wv)
    Wo = np.asarray(Wo, dtype=np.float32)
    for c in range(N_CORES):
        in_maps[c]["wo"] = np.ascontiguousarray(Wo[128 * c : 128 * (c + 1), :])

    res = run_bass_kernel_spmd(
        nc, in_maps, core_ids=list(range(N_CORES)), trace=TRACE
    )
    LAST_RESULT = res

    acc = res.results[0]["y"].astype(np.float64)
    for c in range(1, N_CORES):
        acc = acc + res.results[c]["y"]
    out = acc.astype(np.float32) + np.asarray(bo, dtype=np.float32)[None, :]
    return out.reshape(B, T, C)
